# revision 42
# baseline (speedup 1.0000x reference)
"""Trainium2 Bass kernel for nn_Attention (GQA causal attention + RoPE), v3.

Full problem: x[4,2048,2048] -> attention(16 q heads / 8 kv heads, head_dim
128, llama RoPE, causal) -> out[4,2048,2048], fp32.

Sharding: core = batch*2 + head_group (tensor-parallel over heads x
data-parallel over batch). Host sums the two head-group partials per batch
(the Wo all-reduce).

Per-core kernel, mixed precision tuned to the TRN2 cost model:
- Q/K/V projections: 3-term fp8(e4m3) residual DoubleRow matmuls
  (x ~ xhi+xlo, W ~ whi+wlo; terms hi*hi + hi*lo + lo*hi), 0.75x the
  bf16 cost at ~0.2% error. x scaled by 32, W by 512 on host.
- RoPE: fp16 on DVE (ACT copy-with-scale evacuates PSUM at 1/16384).
- scores: fp16 matmuls into PSUM swaths, NARROWED on the diagonal
  (block di only computes q >= di*128); causal mask ADDED in PSUM by
  tiny fp8 DoubleRow bias matmuls ((-15 I) @ (15 U) = -225).
- softmax: exp on ACT emits fp8(e4m3) probs directly, in a 2-k-block
  paired layout; denominators accumulate the SAME fp8 values (errors
  cancel partially); fp16 reciprocal on DVE.
- PV: fp8 DoubleRow matmuls packing TWO k-blocks (256 contraction) per
  pass; v kept as a x16-scaled fp8 hi/lo pair -> 2 passes per block
  pair = 0.25x the fp16 cost.
- normalize writes 16*attn (fp16) into the dead q slice of rotq; out
  projection converts it lazily per 128-token strip to an fp8 hi/lo
  pair and runs 3-term fp8 DoubleRow matmuls against fp8 hi/lo Wo
  (0.75x fp16); partial outputs written fp16, host sums in fp32.
"""

import math
from contextlib import ExitStack

import numpy as np
import ml_dtypes

import concourse.bass as bass
import concourse.bass_isa as bass_isa
import concourse.mybir as mybir
import concourse.tile as tile
from concourse import bacc
from concourse.bass_utils import run_bass_kernel_spmd

F32 = mybir.dt.float32
FP16 = mybir.dt.float16
FP8 = mybir.dt.float8e4
F8NP = ml_dtypes.float8_e4m3
DR = mybir.MatmulPerfMode.DoubleRow

B, S, D = 4, 2048, 2048
H, KVH, HD = 16, 8, 128
NG = 2
NQ = H // NG           # 8 q heads per core
NKV = KVH // NG        # 4 kv heads per core
REP = NQ // NKV
N_CORES = 8
QW = 512               # q-chunk width
KW = 128               # k-block width
SX, SW = 32.0, 512.0   # host-side fp8 scales for x and W
SV = 16.0              # v (and attn) fp8 scale
SCALE = 1.0 / math.sqrt(HD)
Q2B = True             # 2-term q projection (drop w_lo term)


def _body(nc, tc, cfg, t):
    TOK = cfg["TOK"]
    DM = cfg["DM"]
    DC = DM // 128
    NP_ = DC // 2          # dc pairs
    NTC = TOK // QW        # chunks
    KB = TOK // KW         # k blocks
    KBP = KB // 2          # k block pairs
    inv = 1.0 / (SX * SW)

    with ExitStack() as es:
        P = es.enter_context(tc.tile_pool(name="persist", bufs=1))
        cpk = P.tile([128, TOK], FP16, tag="cpk", name="cpk")
        spk = P.tile([128, TOK], FP16, tag="spk", name="spk")
        negI = P.tile([64, 2, 128], FP8, tag="negI", name="negI")
        umask = P.tile([64, 4, 2, QW], FP8, tag="umask", name="umask")
        rotq = P.tile([128, NQ, TOK], FP16, tag="rotq", name="rotq")
        rotk = P.tile([128, NKV, TOK], FP16, tag="rotk", name="rotk")
        v8h = P.tile([128, KBP, 2, NKV * HD], FP8, tag="v8h", name="v8h")
        v8l = P.tile([128, KBP, 2, NKV * HD], FP8, tag="v8l", name="v8l")

        work = es.enter_context(tc.tile_pool(name="work", bufs=1))
        pp = es.enter_context(tc.tile_pool(name="pp", bufs=1, space="PSUM"))

        # ---- constant loads (cpk first: warmup + rope) ----
        nc.sync.dma_start(out=cpk[:], in_=t["cpk"].ap()[:])
        nc.gpsimd.dma_start(out=spk[:], in_=t["spk"].ap()[:])
        nc.gpsimd.dma_start(out=negI[:], in_=t["negI"].ap()[:])
        nc.gpsimd.dma_start(out=umask[:], in_=t["umask"].ap()[:])

        wts = ExitStack()
        WP = wts.enter_context(tc.tile_pool(name="wts", bufs=1))
        wqh = WP.tile([128, DC, NQ * HD], FP8, tag="wqh", name="wqh")
        if not Q2B:
            wql = WP.tile([128, DC, NQ * HD], FP8, tag="wql", name="wql")
        wkvh = WP.tile([128, DC, 2 * NKV * HD], FP8, tag="wkvh", name="wkvh")
        wkvl = WP.tile([128, DC, 2 * NKV * HD], FP8, tag="wkvl", name="wkvl")

        def load_chunk(c):
            xt = WP.tile([128, DC, 2 * QW], FP8, tag="xth", bufs=2, name="xt")
            ts = c * 2 * QW
            for dc in range(DC):
                eng = (nc.sync, nc.scalar, nc.gpsimd)[dc % 3]
                eng.dma_start(
                    out=xt[:, dc, :],
                    in_=t["x8"].ap()[dc * 128:(dc + 1) * 128,
                                     ts:ts + 2 * QW])
            return xt

        # PE p-state warmup: matmuls on a memset tile while loads run
        wum = pp.tile([128, 2, QW], F32, tag="swath", bufs=2, name="wum")
        wsrc = work.tile([128, 256], FP16, tag="wsrc", bufs=1, name="wsrc")
        nc.vector.memset(wsrc[:], 0.5)
        for i in range(40):
            nc.tensor.matmul(wum[:, 0, 0:128], wsrc[:, 0:128],
                             wsrc[:, 128:256],
                             start=True, stop=True, skip_group_check=True)

        xt0 = WP.tile([128, DC, 2 * QW], FP8, tag="xth", bufs=2, name="xt0")
        for dc in range(DC):
            # spread chunk-0/wkv loads across both HWDGE queues + SWDGE
            e1 = (nc.sync, nc.scalar, nc.gpsimd)[dc % 3]
            e2 = (nc.scalar, nc.gpsimd, nc.sync)[dc % 3]
            e4 = (nc.gpsimd, nc.sync, nc.scalar)[dc % 3]
            e1.dma_start(
                out=xt0[:, dc, :],
                in_=t["x8"].ap()[dc * 128:(dc + 1) * 128, 0:2 * QW])
            e2.dma_start(out=wkvh[:, dc, :],
                         in_=t["wkvh"].ap()[dc * 128:(dc + 1) * 128, :])
            e4.dma_start(
                out=wkvl[:, dc, :],
                in_=t["wkvl"].ap()[dc * 128:(dc + 1) * 128, :])
        for dc in range(DC):
            nc.sync.dma_start(out=wqh[:, dc, :],
                              in_=t["wqh"].ap()[dc * 128:(dc + 1) * 128, :])
            if not Q2B:
                nc.gpsimd.dma_start(
                    out=wql[:, dc, :],
                    in_=t["wql"].ap()[dc * 128:(dc + 1) * 128, :])

        def proj3(ps, wh, wl, xt, coff, cw, xoff=None, xw=None):
            """3-term fp8 residual projection over all dc pairs.

            xt holds [hi | lo] halves packed along the free dim.
            wl=None drops the w_lo term (2-term projection).
            """
            plan = []
            for p in range(NP_):
                dcs = slice(2 * p, 2 * p + 2)
                if xoff is None:
                    plan.append((0, wh[:, dcs, coff:coff + cw],
                                 xt[:, dcs, 0:QW]))
                    if wl is not None:
                        plan.append((1, wl[:, dcs, coff:coff + cw],
                                     xt[:, dcs, 0:QW]))
                    plan.append((1, wh[:, dcs, coff:coff + cw],
                                 xt[:, dcs, QW:2 * QW]))
                else:
                    plan.append((0, xt[:, dcs, xoff:xoff + xw],
                                 wh[:, dcs, coff:coff + cw]))
                    plan.append((1, xt[:, dcs, QW + xoff:QW + xoff + xw],
                                 wh[:, dcs, coff:coff + cw]))
                    if wl is not None:
                        plan.append((1, xt[:, dcs, xoff:xoff + xw],
                                     wl[:, dcs, coff:coff + cw]))
            plan.sort(key=lambda it: it[0])  # all hi*hi terms first
            for i, (_, a, b_) in enumerate(plan):
                nc.tensor.matmul(ps, a, b_, start=(i == 0),
                                 stop=(i == len(plan) - 1), perf_mode=DR,
                                 skip_group_check=True)

        NH = NQ + NKV  # heads per chunk-batch of rope work (12)
        NHB = NH // 6  # rope dup processed in 6 head-batches of 2

        def proj_psum():
            tl = pp.tile([128, 2, QW], F32, tag="swath", bufs=2, name="ps")
            return tl[:, 0, :]

        def proj_head(wh, wl, coff, rawAll, hh, xt):
            """Project one q/k head; evacuate into rawAll[:, hh, :]."""
            ps = proj_psum()
            proj3(ps, wh, wl, xt, coff, HD)
            nc.scalar.activation(rawAll[:, hh, :], ps,
                                 mybir.ActivationFunctionType.Copy, scale=inv)

        def rope_batch(rawAll, qeAll, qoAll, dests, ts):
            """Duplicate even/odd halves for a head-batch, then rotate."""
            nc.gpsimd.dma_start(out=qeAll[0:64, :, :], in_=rawAll[0:64, :, :])
            nc.sync.dma_start(out=qeAll[64:128, :, :],
                               in_=rawAll[0:64, :, :])
            nc.gpsimd.dma_start(out=qoAll[0:64, :, :],
                                in_=rawAll[64:128, :, :])
            nc.sync.dma_start(out=qoAll[64:128, :, :],
                               in_=rawAll[64:128, :, :])
            for i, dest in enumerate(dests):
                t1 = work.tile([128, QW], FP16, tag="t1", bufs=1, name="t1")
                t2 = work.tile([128, QW], FP16, tag="t2", bufs=1, name="t2")
                nc.vector.tensor_mul(t1[:], qeAll[:, i, :], cpk[:, ts:ts + QW])
                nc.vector.tensor_mul(t2[:], qoAll[:, i, :], spk[:, ts:ts + QW])
                nc.vector.tensor_add(dest, t1[:], t2[:])

        # ---- wave A: chunks 0..NTC-2; last chunk interleaves into
        #      the ACT-bound early attention as PE filler ----
        def chunk_units(c, xt):
            """Generate per-chunk projection work as callable units."""
            ts = c * QW
            vunits = []
            for tb in range(QW // KW):
                def vproj(tb=tb):
                    psv = proj_psum()
                    proj3(psv, wkvh, wkvl, xt, NKV * HD, NKV * HD,
                          xoff=tb * KW, xw=KW)
                    blk = c * (QW // KW) + tb
                    v16 = work.tile([128, NKV * HD], FP16, tag="v16",
                                    bufs=1, name="v16")
                    nc.vector.tensor_scalar_mul(v16[:], psv, inv * SV)
                    nc.vector.tensor_copy(v8h[:, blk // 2, blk % 2, :],
                                          v16[:])
                    nc.vector.tensor_sub(v8l[:, blk // 2, blk % 2, :],
                                         v16[:], v8h[:, blk // 2, blk % 2, :])
                vunits.append(vproj)
            units = []
            heads = ([("kv", kv) for kv in range(NKV)]
                     + [("q", h) for h in range(NQ)])
            for hb in range(6):
                def ropeu(hb=hb):
                    batch = heads[hb * NHB:(hb + 1) * NHB]
                    rawAll = work.tile([128, NHB, QW], FP16, tag="rawAll",
                                       bufs=2, name="rawAll")
                    qeAll = work.tile([128, NHB, QW], FP16, tag="qeAll",
                                      bufs=2, name="qeAll")
                    qoAll = work.tile([128, NHB, QW], FP16, tag="qoAll",
                                      bufs=2, name="qoAll")
                    dests = []
                    for i, (kind, idx) in enumerate(batch):
                        if kind == "kv":
                            proj_head(wkvh, wkvl, idx * HD, rawAll, i, xt)
                            dests.append(rotk[:, idx, ts:ts + QW])
                        else:
                            proj_head(wqh, None if Q2B else wql, idx * HD,
                                      rawAll, i, xt)
                            dests.append(rotq[:, idx, ts:ts + QW])
                    rope_batch(rawAll, qeAll, qoAll, dests, ts)
                units.append(ropeu)
            # k/q head units first (attention-critical), v projections last
            return units + vunits




        # -------- attention by head-pair, j outer; outproj interleaved ----
        # k blocks are processed in PAIRS (DoubleRow 256-deep PV contraction)
        # all-ones stationary: the denominator matmul broadcasts the k-sum
        # of e to every output partition (no partition reduce needed)
        ones8 = P.tile([128, 2, 128], FP8, tag="ones8", name="ones8")
        nc.vector.memset(ones8[:], 1.0)
        pend = []

        def _finish(pr, j, poP, denP):
            qs = j * QW
            bc = work.tile([128, 2, QW], FP16, tag="bc", bufs=1, name="bc")
            with nc.allow_low_precision(reason="softmax rec fp16"):
                nc.vector.reciprocal(bc[:], denP[:])
            # writes 16*attn into the dead q slice of rotq (v was x16)
            nc.vector.tensor_mul(rotq[:, 2 * pr:2 * pr + 2, qs:qs + QW],
                                 poP[:], bc[:])

        def flush_pend(depth=1):
            while len(pend) > depth:
                _flush_one()

        def _flush_one():
            e, pr, j, p, po_off, poP, denP = pend.pop(0)
            kh = pr
            npair = 2 * j + 2
            st, sp = (p == 0), (p == npair - 1)
            for i in range(2):
                for ti, v8 in ((0, v8h), (1, v8l)):
                    nc.tensor.matmul(
                        poP[:, i, po_off:],
                        v8[:, p, :, kh * HD:(kh + 1) * HD],
                        e[:, :, i, po_off:],
                        start=(st and ti == 0), stop=(sp and ti == 1),
                        perf_mode=DR, skip_group_check=True)
                # softmax denominator: ones.T @ e on the PE (fp8 DoubleRow)
                nc.tensor.matmul(
                    denP[:, i, po_off:], ones8[:], e[:, :, i, po_off:],
                    start=st, stop=sp, perf_mode=DR, skip_group_check=True)
            if sp:
                _finish(pr, j, poP, denP)

        def attn_pair(pr, j, fillers=(), foff=0):
            kh = pr
            qs = j * QW
            fillers = list(fillers)
            npair = 2 * j + 2
            nfp = npair - foff
            poP = eaP = None
            for p in range(2 * j + 2):
                # PE filler while ACT chews on exp: spread over pairs >= foff
                if p >= foff:
                    lo = (p - foff) * len(fillers) // nfp
                    hi = (p - foff + 1) * len(fillers) // nfp
                    for u in fillers[lo:hi]:
                        u()
                b0 = 2 * p
                # pair q-range starts at the even block's diag offset
                po_off = max(0, (b0 - 4 * j)) * KW
                swts = []
                for s in range(2):
                    b = b0 + s
                    diag = b >= 4 * j
                    ob = max(0, (b - 4 * j)) * KW
                    swt = pp.tile([128, 2, QW], F32, tag="swath", bufs=2,
                                  name="swt")
                    swts.append(swt)
                    strip = diag and ob > po_off
                    for i in range(2):
                        if strip:
                            # fully-masked strip left of this block's own
                            # range: constant -225. Must be emitted FIRST:
                            # start=True lazily zeroes the whole 2KB bank.
                            nc.tensor.matmul(
                                swt[:, i, po_off:ob], negI[:],
                                umask[:, b - 4 * j, :, po_off:ob],
                                start=True, stop=False,
                                perf_mode=DR, skip_group_check=True)
                        nc.tensor.matmul(
                            swt[:, i, ob:],
                            rotk[:, kh, b * 128:(b + 1) * 128],
                            rotq[:, 2 * pr + i, qs + ob:qs + QW],
                            start=not strip, stop=not diag,
                            skip_group_check=True)
                    if diag:
                        di = b - 4 * j
                        for i in range(2):
                            nc.tensor.matmul(
                                swt[:, i, ob:ob + KW], negI[:],
                                umask[:, di, :, ob:ob + KW],
                                start=False, stop=True,
                                perf_mode=DR, skip_group_check=True)
                e = work.tile([128, 2, 2, QW], FP8, tag="e", bufs=4,
                              name="e")
                for s in range(2):
                    nc.scalar.activation(e[:, s, :, po_off:],
                                         swts[s][:, :, po_off:],
                                         mybir.ActivationFunctionType.Exp,
                                         scale=SCALE)
                flush_pend(depth=2)
                if p == 0:
                    poP = pp.tile([128, 2, QW], F32, tag="popair", bufs=1,
                                  name="poP")
                    denP = pp.tile([128, 2, QW], F32, tag="den", bufs=1,
                                   name="denP")
                pend.append((e, pr, j, p, po_off, poP, denP))

        _last_tb = [-1, None]

        def outproj_units(units):
            for tb, oc in units:
                tsl = slice(tb * KW, (tb + 1) * KW)
                if tb != _last_tb[0]:
                    # convert this token strip of 16*attn to an fp8 pair
                    a8h = work.tile([128, NQ, KW], FP8, tag="a8h", bufs=2,
                                    name="a8h")
                    a8l = work.tile([128, NQ, KW], FP8, tag="a8l", bufs=2,
                                    name="a8l")
                    nc.vector.tensor_copy(a8h[:], rotq[:, :, tsl])
                    nc.vector.tensor_sub(a8l[:], rotq[:, :, tsl], a8h[:])
                    _last_tb[0] = tb
                    _last_tb[1] = (a8h, a8l)
                a8h, a8l = _last_tb[1]
                ots = pp.tile([128, 2, QW], F32, tag="swath", bufs=2,
                              name="ots")
                ot = ots[:, 0, :]
                osl = slice(oc * QW, (oc + 1) * QW)
                nmm = 3 * (NQ // 2)
                mi = 0
                for hp in range(NQ // 2):
                    hs = slice(2 * hp, 2 * hp + 2)
                    for a8, wo8 in ((a8h, wo8h), (a8h, wo8l), (a8l, wo8h)):
                        nc.tensor.matmul(ot, a8[:, hs, :], wo8[:, hs, osl],
                                         start=(mi == 0),
                                         stop=(mi == nmm - 1),
                                         perf_mode=DR,
                                         skip_group_check=True)
                        mi += 1
                osb = work.tile([128, QW], FP16, tag="osb", bufs=2,
                                name="osb")
                oscale = 1.0 / (SV * SW)
                # DVE evac: ACT is the bottleneck during the attention phase
                nc.vector.tensor_scalar_mul(osb[:], ot, oscale)
                dma_eng = nc.sync if (tb + oc) % 2 == 0 else nc.scalar
                dma_eng.dma_start(
                    out=t["out"].ap()[tb * KW:(tb + 1) * KW,
                                      oc * QW:(oc + 1) * QW],
                    in_=osb[:])

        NOC = DM // QW

        def oust(j):
            return [(tb, oc) for tb in range(j * 4, (j + 1) * 4)
                    for oc in range(NOC)]

        # ---- diagonal pipeline: chunk-c projections interleave with
        #      attention for chunk c-1 (uses wave-A ACT slack for exp) ----
        for c in range(NTC):
            xtc = xt0 if c == 0 else load_chunk(c)
            units = chunk_units(c, xtc)
            if c == 0:
                for u in units:
                    u()
            else:
                n = len(units)
                for pr in range(NKV):
                    lo = pr * n // NKV
                    hi = (pr + 1) * n // NKV
                    attn_pair(pr, c - 1, fillers=units[lo:hi])
        # weights done; free their SBUF, then load Wo (fp8 hi/lo pair)
        wts.close()
        late = es.enter_context(tc.tile_pool(name="late", bufs=1))
        wo8h = late.tile([128, NQ, DM], FP8, tag="wo8h", name="wo8h")
        wo8l = late.tile([128, NQ, DM], FP8, tag="wo8l", name="wo8l")
        for h in range(NQ):
            eng = nc.sync if h % 2 == 0 else nc.scalar
            eng.dma_start(out=wo8h[:, h, :],
                          in_=t["wo8h"].ap()[h * HD:(h + 1) * HD, :])
            eng2 = nc.scalar if h % 2 == 0 else nc.sync
            eng2.dma_start(out=wo8l[:, h, :],
                           in_=t["wo8l"].ap()[h * HD:(h + 1) * HD, :])
        # j3 attention with outproj of chunks 0-2 as PE filler
        fill = oust(0) + oust(1) + oust(2)
        for pr in range(NKV):
            mine = fill[pr * 12:(pr + 1) * 12]
            attn_pair(pr, 3, fillers=[
                (lambda u=u: outproj_units([u])) for u in mine])
        flush_pend(depth=0)
        outproj_units(oust(3))


def build(TOK=S, DM=D):
    cfg = dict(TOK=TOK, DM=DM)
    nc = bacc.Bacc("TRN2", target_bir_lowering=False, debug=False)
    t = {}
    t["x8"] = nc.dram_tensor("x8", [DM, 2 * TOK], FP8, kind="ExternalInput")
    t["wqh"] = nc.dram_tensor("wqh", [DM, NQ * HD], FP8, kind="ExternalInput")
    if not Q2B:
        t["wql"] = nc.dram_tensor("wql", [DM, NQ * HD], FP8,
                                  kind="ExternalInput")
    t["wkvh"] = nc.dram_tensor("wkvh", [DM, 2 * NKV * HD], FP8,
                               kind="ExternalInput")
    t["wkvl"] = nc.dram_tensor("wkvl", [DM, 2 * NKV * HD], FP8,
                               kind="ExternalInput")
    t["wo8h"] = nc.dram_tensor("wo8h", [NQ * HD, DM], FP8,
                               kind="ExternalInput")
    t["wo8l"] = nc.dram_tensor("wo8l", [NQ * HD, DM], FP8,
                               kind="ExternalInput")
    t["cpk"] = nc.dram_tensor("cpk", [128, TOK], FP16, kind="ExternalInput")
    t["spk"] = nc.dram_tensor("spk", [128, TOK], FP16, kind="ExternalInput")
    t["negI"] = nc.dram_tensor("negI", [64, 2 * 128], FP8,
                               kind="ExternalInput")
    t["umask"] = nc.dram_tensor("umask", [64, 4 * 2 * QW], FP8,
                                kind="ExternalInput")
    t["out"] = nc.dram_tensor("out", [TOK, DM], FP16, kind="ExternalOutput")
    with tile.TileContext(nc) as tc:
        _body(nc, tc, cfg, t)
    nc.compile()
    return nc


# ---------------- host-side sharding ----------------

def _rope_perm():
    return np.concatenate([np.arange(0, 128, 2), np.arange(1, 128, 2)])


def _res(v):
    hi = v.astype(F8NP)
    lo = (v - hi.astype(np.float32)).astype(F8NP)
    return hi, lo


def _consts():
    negI = np.zeros((64, 2, 128), np.float32)
    for sl in range(2):
        for r in range(64):
            negI[r, sl, r + 64 * sl] = -15.0
    kk = np.arange(128)[:, None]
    qq = np.arange(QW)[None, :]
    umask = np.zeros((64, 4, 2, QW), np.float32)
    for di in range(4):
        u = ((di * 128 + kk) > qq) * 15.0
        umask[:, di, 0, :] = u[0:64]
        umask[:, di, 1, :] = u[64:128]
    return (negI.reshape(64, 256).astype(F8NP),
            umask.reshape(64, 4 * 2 * QW).astype(F8NP))


def shard_inputs(x, freqs_cos, freqs_sin, Wq, Wk, Wv, Wo):
    perm = _rope_perm()
    negI, umask = _consts()
    cpk = np.concatenate([freqs_cos.T, freqs_sin.T], 0).astype(np.float16)
    spk = np.concatenate([-freqs_sin.T, freqs_cos.T], 0).astype(np.float16)

    in_maps = []
    for b in range(B):
        xt = np.ascontiguousarray(np.asarray(x)[b].T).astype(np.float32) * SX
        xhi, xlo = _res(xt)
        x8 = np.empty((D, 2 * S), F8NP)
        for c in range(S // QW):
            x8[:, c * 2 * QW:c * 2 * QW + QW] = xhi[:, c * QW:(c + 1) * QW]
            x8[:, c * 2 * QW + QW:(c + 1) * 2 * QW] = (
                xlo[:, c * QW:(c + 1) * QW])
        for g in range(NG):
            qh = slice(g * NQ * HD, (g + 1) * NQ * HD)
            kvh = slice(g * NKV * HD, (g + 1) * NKV * HD)
            wq_g = (Wq[:, qh].reshape(D, NQ, HD)[:, :, perm]
                    .reshape(D, NQ * HD).astype(np.float32) * SW)
            wk_g = (Wk[:, kvh].reshape(D, NKV, HD)[:, :, perm]
                    .reshape(D, NKV * HD).astype(np.float32) * SW)
            wkv = np.concatenate([wk_g, Wv[:, kvh].astype(np.float32) * SW],
                                 axis=1)
            wqh_, wql_ = _res(np.ascontiguousarray(wq_g))
            wkvh_, wkvl_ = _res(np.ascontiguousarray(wkv))
            wo8h_, wo8l_ = _res(
                np.ascontiguousarray(Wo[qh, :]).astype(np.float32) * SW)
            im = dict(
                x8=x8, wqh=wqh_, wkvh=wkvh_, wkvl=wkvl_,
                wo8h=wo8h_, wo8l=wo8l_,
                cpk=cpk, spk=spk, negI=negI, umask=umask,
            )
            if not Q2B:
                im["wql"] = wql_
            in_maps.append(im)
    return in_maps


_NC_CACHE = {}


def kernel(x, freqs_cos, freqs_sin, Wq, Wk, Wv, Wo):
    """Full-problem entry point: full inputs in, full [B,S,D] fp32 out."""
    if "nc" not in _NC_CACHE:
        _NC_CACHE["nc"] = build()
    nc = _NC_CACHE["nc"]
    in_maps = shard_inputs(
        np.asarray(x), np.asarray(freqs_cos), np.asarray(freqs_sin),
        np.asarray(Wq), np.asarray(Wk), np.asarray(Wv), np.asarray(Wo),
    )
    res = run_bass_kernel_spmd(nc, in_maps, core_ids=list(range(N_CORES)))
    out = np.zeros((B, S, D), np.float32)
    for b in range(B):
        out[b] = (res.results[b * NG]["out"].astype(np.float32)
                  + res.results[b * NG + 1]["out"].astype(np.float32))
    return out


# revision 44
# speedup vs baseline: 1.0012x; 1.0012x over previous
"""Trainium2 Bass kernel for nn_Attention (GQA causal attention + RoPE), v3.

Full problem: x[4,2048,2048] -> attention(16 q heads / 8 kv heads, head_dim
128, llama RoPE, causal) -> out[4,2048,2048], fp32.

Sharding: core = batch*2 + head_group (tensor-parallel over heads x
data-parallel over batch). Host sums the two head-group partials per batch
(the Wo all-reduce).

Per-core kernel, mixed precision tuned to the TRN2 cost model:
- K/V projections: 3-term fp8(e4m3) residual DoubleRow matmuls
  (x ~ xhi+xlo, W ~ whi+wlo; terms hi*hi + hi*lo + lo*hi), 0.75x the
  bf16 cost. Q projection: 2-term (w_lo dropped, Q2B). x scaled by 32,
  W by 512 on host.
- RoPE: fp16 on DVE (ACT copy-with-scale evacuates PSUM at 1/16384).
- scores: fp16 matmuls into PSUM swaths, NARROWED on the diagonal
  (block di only computes q >= di*128); causal mask ADDED in PSUM by
  tiny fp8 DoubleRow bias matmuls ((-15 I) @ (15 U) = -225); fully
  masked strips are written by a start=True mask matmul emitted FIRST
  (start=True lazily zeroes the whole 2KB bank).
- softmax: exp on ACT emits fp8(e4m3) probs directly, in a 2-k-block
  paired layout; the denominator is an all-ones [128,2,128] fp8
  DoubleRow matmul on the PE (k-sum broadcast to all partitions, no
  partition reduce); fp16 reciprocal on DVE.
- PV: fp8 DoubleRow matmuls packing TWO k-blocks (256 contraction) per
  pass; v kept as a x16-scaled fp8 hi/lo pair -> 2 passes per block
  pair = 0.5x the fp16 cost (0.25x with narrowing).
- normalize writes 16*attn (fp16) into the dead q slice of rotq; out
  projection converts it lazily per 128-token strip to an fp8 hi/lo
  pair and runs 3-term fp8 DoubleRow matmuls against fp8 hi/lo Wo
  (0.75x fp16); partial outputs written fp16, host sums in fp32.
- schedule: diagonal pipeline — chunk-c projections run as PE fillers
  inside the (ACT-bound) attention pairs of chunk c-1; j3 attention is
  filled with out-projection units of chunks 0-2.
"""

import math
from contextlib import ExitStack

import numpy as np
import ml_dtypes

import concourse.bass as bass
import concourse.bass_isa as bass_isa
import concourse.mybir as mybir
import concourse.tile as tile
from concourse import bacc
from concourse.bass_utils import run_bass_kernel_spmd

F32 = mybir.dt.float32
FP16 = mybir.dt.float16
FP8 = mybir.dt.float8e4
F8NP = ml_dtypes.float8_e4m3
DR = mybir.MatmulPerfMode.DoubleRow

B, S, D = 4, 2048, 2048
H, KVH, HD = 16, 8, 128
NG = 2
NQ = H // NG           # 8 q heads per core
NKV = KVH // NG        # 4 kv heads per core
REP = NQ // NKV
N_CORES = 8
QW = 512               # q-chunk width
KW = 128               # k-block width
SX, SW = 32.0, 512.0   # host-side fp8 scales for x and W
SV = 16.0              # v (and attn) fp8 scale
SCALE = 1.0 / math.sqrt(HD)
Q2B = True             # 2-term q projection (drop w_lo term)


def _body(nc, tc, cfg, t):
    TOK = cfg["TOK"]
    DM = cfg["DM"]
    DC = DM // 128
    NP_ = DC // 2          # dc pairs
    NTC = TOK // QW        # chunks
    KB = TOK // KW         # k blocks
    KBP = KB // 2          # k block pairs
    inv = 1.0 / (SX * SW)

    with ExitStack() as es:
        P = es.enter_context(tc.tile_pool(name="persist", bufs=1))
        cpk = P.tile([128, TOK], FP16, tag="cpk", name="cpk")
        spk = P.tile([128, TOK], FP16, tag="spk", name="spk")
        negI = P.tile([64, 2, 128], FP8, tag="negI", name="negI")
        umask = P.tile([64, 4, 2, QW], FP8, tag="umask", name="umask")
        rotq = P.tile([128, NQ, TOK], FP16, tag="rotq", name="rotq")
        rotk = P.tile([128, NKV, TOK], FP16, tag="rotk", name="rotk")
        v8h = P.tile([128, KBP, 2, NKV * HD], FP8, tag="v8h", name="v8h")
        v8l = P.tile([128, KBP, 2, NKV * HD], FP8, tag="v8l", name="v8l")

        work = es.enter_context(tc.tile_pool(name="work", bufs=1))
        pp = es.enter_context(tc.tile_pool(name="pp", bufs=1, space="PSUM"))

        # ---- constant loads (cpk first: warmup + rope) ----
        nc.sync.dma_start(out=cpk[:], in_=t["cpk"].ap()[:])
        nc.scalar.dma_start(out=spk[:], in_=t["spk"].ap()[:])
        nc.scalar.dma_start(out=negI[:], in_=t["negI"].ap()[:])
        nc.scalar.dma_start(out=umask[:], in_=t["umask"].ap()[:])

        wts = ExitStack()
        WP = wts.enter_context(tc.tile_pool(name="wts", bufs=1))
        wqh = WP.tile([128, DC, NQ * HD], FP8, tag="wqh", name="wqh")
        if not Q2B:
            wql = WP.tile([128, DC, NQ * HD], FP8, tag="wql", name="wql")
        wkvh = WP.tile([128, DC, 2 * NKV * HD], FP8, tag="wkvh", name="wkvh")
        wkvl = WP.tile([128, DC, 2 * NKV * HD], FP8, tag="wkvl", name="wkvl")

        def load_chunk(c):
            xt = WP.tile([128, DC, 2 * QW], FP8, tag="xth", bufs=2, name="xt")
            ts = c * 2 * QW
            for dc in range(DC):
                eng = (nc.sync, nc.scalar, nc.gpsimd)[dc % 3]
                eng.dma_start(
                    out=xt[:, dc, :],
                    in_=t["x8"].ap()[dc * 128:(dc + 1) * 128,
                                     ts:ts + 2 * QW])
            return xt

        # PE p-state warmup: matmuls on a memset tile while loads run
        wum = pp.tile([128, 2, QW], F32, tag="swath", bufs=2, name="wum")
        wsrc = work.tile([128, 256], FP16, tag="wsrc", bufs=1, name="wsrc")
        nc.vector.memset(wsrc[:], 0.5)
        for i in range(40):
            nc.tensor.matmul(wum[:, 0, 0:128], wsrc[:, 0:128],
                             wsrc[:, 128:256],
                             start=True, stop=True, skip_group_check=True)

        xt0 = WP.tile([128, DC, 2 * QW], FP8, tag="xth", bufs=2, name="xt0")
        for dc in range(DC):
            # spread chunk-0/wkv loads across both HWDGE queues + SWDGE
            e1 = (nc.sync, nc.scalar, nc.gpsimd)[dc % 3]
            e2 = (nc.scalar, nc.gpsimd, nc.sync)[dc % 3]
            e4 = (nc.gpsimd, nc.sync, nc.scalar)[dc % 3]
            e1.dma_start(
                out=xt0[:, dc, :],
                in_=t["x8"].ap()[dc * 128:(dc + 1) * 128, 0:2 * QW])
            e2.dma_start(out=wkvh[:, dc, :],
                         in_=t["wkvh"].ap()[dc * 128:(dc + 1) * 128, :])
            e4.dma_start(
                out=wkvl[:, dc, :],
                in_=t["wkvl"].ap()[dc * 128:(dc + 1) * 128, :])
        for dc in range(DC):
            nc.sync.dma_start(out=wqh[:, dc, :],
                              in_=t["wqh"].ap()[dc * 128:(dc + 1) * 128, :])
            if not Q2B:
                nc.gpsimd.dma_start(
                    out=wql[:, dc, :],
                    in_=t["wql"].ap()[dc * 128:(dc + 1) * 128, :])

        def proj3(ps, wh, wl, xt, coff, cw, xoff=None, xw=None):
            """3-term fp8 residual projection over all dc pairs.

            xt holds [hi | lo] halves packed along the free dim.
            wl=None drops the w_lo term (2-term projection).
            """
            plan = []
            for p in range(NP_):
                dcs = slice(2 * p, 2 * p + 2)
                if xoff is None:
                    plan.append((0, wh[:, dcs, coff:coff + cw],
                                 xt[:, dcs, 0:QW]))
                    if wl is not None:
                        plan.append((1, wl[:, dcs, coff:coff + cw],
                                     xt[:, dcs, 0:QW]))
                    plan.append((1, wh[:, dcs, coff:coff + cw],
                                 xt[:, dcs, QW:2 * QW]))
                else:
                    plan.append((0, xt[:, dcs, xoff:xoff + xw],
                                 wh[:, dcs, coff:coff + cw]))
                    plan.append((1, xt[:, dcs, QW + xoff:QW + xoff + xw],
                                 wh[:, dcs, coff:coff + cw]))
                    if wl is not None:
                        plan.append((1, xt[:, dcs, xoff:xoff + xw],
                                     wl[:, dcs, coff:coff + cw]))
            plan.sort(key=lambda it: it[0])  # all hi*hi terms first
            for i, (_, a, b_) in enumerate(plan):
                nc.tensor.matmul(ps, a, b_, start=(i == 0),
                                 stop=(i == len(plan) - 1), perf_mode=DR,
                                 skip_group_check=True)

        NH = NQ + NKV  # heads per chunk-batch of rope work (12)
        NHB = NH // 6  # rope dup processed in 6 head-batches of 2

        def proj_psum():
            tl = pp.tile([128, 2, QW], F32, tag="swath", bufs=2, name="ps")
            return tl[:, 0, :]

        def proj_head(wh, wl, coff, rawAll, hh, xt):
            """Project one q/k head; evacuate into rawAll[:, hh, :]."""
            ps = proj_psum()
            proj3(ps, wh, wl, xt, coff, HD)
            nc.scalar.activation(rawAll[:, hh, :], ps,
                                 mybir.ActivationFunctionType.Copy, scale=inv)

        def rope_batch(rawAll, qeAll, qoAll, dests, ts):
            """Duplicate even/odd halves for a head-batch, then rotate."""
            nc.gpsimd.dma_start(out=qeAll[0:64, :, :], in_=rawAll[0:64, :, :])
            nc.sync.dma_start(out=qeAll[64:128, :, :],
                               in_=rawAll[0:64, :, :])
            nc.gpsimd.dma_start(out=qoAll[0:64, :, :],
                                in_=rawAll[64:128, :, :])
            nc.sync.dma_start(out=qoAll[64:128, :, :],
                               in_=rawAll[64:128, :, :])
            for i, dest in enumerate(dests):
                t1 = work.tile([128, QW], FP16, tag="t1", bufs=1, name="t1")
                t2 = work.tile([128, QW], FP16, tag="t2", bufs=1, name="t2")
                nc.vector.tensor_mul(t1[:], qeAll[:, i, :], cpk[:, ts:ts + QW])
                nc.vector.tensor_mul(t2[:], qoAll[:, i, :], spk[:, ts:ts + QW])
                nc.vector.tensor_add(dest, t1[:], t2[:])

        # ---- wave A: chunks 0..NTC-2; last chunk interleaves into
        #      the ACT-bound early attention as PE filler ----
        def chunk_units(c, xt):
            """Generate per-chunk projection work as callable units."""
            ts = c * QW
            vunits = []
            for tb in range(QW // KW):
                def vproj(tb=tb):
                    psv = proj_psum()
                    proj3(psv, wkvh, wkvl, xt, NKV * HD, NKV * HD,
                          xoff=tb * KW, xw=KW)
                    blk = c * (QW // KW) + tb
                    v16 = work.tile([128, NKV * HD], FP16, tag="v16",
                                    bufs=1, name="v16")
                    nc.vector.tensor_scalar_mul(v16[:], psv, inv * SV)
                    nc.vector.tensor_copy(v8h[:, blk // 2, blk % 2, :],
                                          v16[:])
                    nc.vector.tensor_sub(v8l[:, blk // 2, blk % 2, :],
                                         v16[:], v8h[:, blk // 2, blk % 2, :])
                vunits.append(vproj)
            units = []
            heads = ([("kv", kv) for kv in range(NKV)]
                     + [("q", h) for h in range(NQ)])
            for hb in range(6):
                def ropeu(hb=hb):
                    batch = heads[hb * NHB:(hb + 1) * NHB]
                    rawAll = work.tile([128, NHB, QW], FP16, tag="rawAll",
                                       bufs=2, name="rawAll")
                    qeAll = work.tile([128, NHB, QW], FP16, tag="qeAll",
                                      bufs=2, name="qeAll")
                    qoAll = work.tile([128, NHB, QW], FP16, tag="qoAll",
                                      bufs=2, name="qoAll")
                    dests = []
                    for i, (kind, idx) in enumerate(batch):
                        if kind == "kv":
                            proj_head(wkvh, wkvl, idx * HD, rawAll, i, xt)
                            dests.append(rotk[:, idx, ts:ts + QW])
                        else:
                            proj_head(wqh, None if Q2B else wql, idx * HD,
                                      rawAll, i, xt)
                            dests.append(rotq[:, idx, ts:ts + QW])
                    rope_batch(rawAll, qeAll, qoAll, dests, ts)
                units.append(ropeu)
            # k/q head units first (attention-critical), v projections last
            return units + vunits




        # -------- attention by head-pair, j outer; outproj interleaved ----
        # k blocks are processed in PAIRS (DoubleRow 256-deep PV contraction)
        # all-ones stationary: the denominator matmul broadcasts the k-sum
        # of e to every output partition (no partition reduce needed)
        ones8 = P.tile([128, 2, 128], FP8, tag="ones8", name="ones8")
        nc.vector.memset(ones8[:], 1.0)
        pend = []

        def _finish(pr, j, poP, denP):
            qs = j * QW
            bc = work.tile([128, 2, QW], FP16, tag="bc", bufs=1, name="bc")
            with nc.allow_low_precision(reason="softmax rec fp16"):
                nc.vector.reciprocal(bc[:], denP[:])
            # writes 16*attn into the dead q slice of rotq (v was x16)
            nc.vector.tensor_mul(rotq[:, 2 * pr:2 * pr + 2, qs:qs + QW],
                                 poP[:], bc[:])

        def flush_pend(depth=1):
            while len(pend) > depth:
                _flush_one()

        def _flush_one():
            e, pr, j, p, po_off, poP, denP = pend.pop(0)
            kh = pr
            npair = 2 * j + 2
            st, sp = (p == 0), (p == npair - 1)
            for i in range(2):
                for ti, v8 in ((0, v8h), (1, v8l)):
                    nc.tensor.matmul(
                        poP[:, i, po_off:],
                        v8[:, p, :, kh * HD:(kh + 1) * HD],
                        e[:, :, i, po_off:],
                        start=(st and ti == 0), stop=(sp and ti == 1),
                        perf_mode=DR, skip_group_check=True)
                # softmax denominator: ones.T @ e on the PE (fp8 DoubleRow)
                nc.tensor.matmul(
                    denP[:, i, po_off:], ones8[:], e[:, :, i, po_off:],
                    start=st, stop=sp, perf_mode=DR, skip_group_check=True)
            if sp:
                _finish(pr, j, poP, denP)

        def attn_pair(pr, j, fillers=(), foff=0):
            kh = pr
            qs = j * QW
            fillers = list(fillers)
            npair = 2 * j + 2
            nfp = npair - foff
            poP = eaP = None
            for p in range(2 * j + 2):
                # PE filler while ACT chews on exp: spread over pairs >= foff
                if p >= foff:
                    lo = (p - foff) * len(fillers) // nfp
                    hi = (p - foff + 1) * len(fillers) // nfp
                    for u in fillers[lo:hi]:
                        u()
                b0 = 2 * p
                # pair q-range starts at the even block's diag offset
                po_off = max(0, (b0 - 4 * j)) * KW
                swts = []
                for s in range(2):
                    b = b0 + s
                    diag = b >= 4 * j
                    ob = max(0, (b - 4 * j)) * KW
                    swt = pp.tile([128, 2, QW], F32, tag="swath", bufs=2,
                                  name="swt")
                    swts.append(swt)
                    strip = diag and ob > po_off
                    for i in range(2):
                        if strip:
                            # fully-masked strip left of this block's own
                            # range: constant -225. Must be emitted FIRST:
                            # start=True lazily zeroes the whole 2KB bank.
                            nc.tensor.matmul(
                                swt[:, i, po_off:ob], negI[:],
                                umask[:, b - 4 * j, :, po_off:ob],
                                start=True, stop=False,
                                perf_mode=DR, skip_group_check=True)
                        nc.tensor.matmul(
                            swt[:, i, ob:],
                            rotk[:, kh, b * 128:(b + 1) * 128],
                            rotq[:, 2 * pr + i, qs + ob:qs + QW],
                            start=not strip, stop=not diag,
                            skip_group_check=True)
                    if diag:
                        di = b - 4 * j
                        for i in range(2):
                            nc.tensor.matmul(
                                swt[:, i, ob:ob + KW], negI[:],
                                umask[:, di, :, ob:ob + KW],
                                start=False, stop=True,
                                perf_mode=DR, skip_group_check=True)
                e = work.tile([128, 2, 2, QW], FP8, tag="e", bufs=4,
                              name="e")
                for s in range(2):
                    nc.scalar.activation(e[:, s, :, po_off:],
                                         swts[s][:, :, po_off:],
                                         mybir.ActivationFunctionType.Exp,
                                         scale=SCALE)
                flush_pend(depth=2)
                if p == 0:
                    poP = pp.tile([128, 2, QW], F32, tag="popair", bufs=1,
                                  name="poP")
                    denP = pp.tile([128, 2, QW], F32, tag="den", bufs=1,
                                   name="denP")
                pend.append((e, pr, j, p, po_off, poP, denP))

        _last_tb = [-1, None]

        def outproj_units(units):
            for tb, oc in units:
                tsl = slice(tb * KW, (tb + 1) * KW)
                if tb != _last_tb[0]:
                    # convert this token strip of 16*attn to an fp8 pair
                    a8h = work.tile([128, NQ, KW], FP8, tag="a8h", bufs=2,
                                    name="a8h")
                    a8l = work.tile([128, NQ, KW], FP8, tag="a8l", bufs=2,
                                    name="a8l")
                    nc.vector.tensor_copy(a8h[:], rotq[:, :, tsl])
                    nc.vector.tensor_sub(a8l[:], rotq[:, :, tsl], a8h[:])
                    _last_tb[0] = tb
                    _last_tb[1] = (a8h, a8l)
                a8h, a8l = _last_tb[1]
                ots = pp.tile([128, 2, QW], F32, tag="swath", bufs=2,
                              name="ots")
                ot = ots[:, 0, :]
                osl = slice(oc * QW, (oc + 1) * QW)
                nmm = 3 * (NQ // 2)
                mi = 0
                for hp in range(NQ // 2):
                    hs = slice(2 * hp, 2 * hp + 2)
                    for a8, wo8 in ((a8h, wo8h), (a8h, wo8l), (a8l, wo8h)):
                        nc.tensor.matmul(ot, a8[:, hs, :], wo8[:, hs, osl],
                                         start=(mi == 0),
                                         stop=(mi == nmm - 1),
                                         perf_mode=DR,
                                         skip_group_check=True)
                        mi += 1
                osb = work.tile([128, QW], FP16, tag="osb", bufs=2,
                                name="osb")
                oscale = 1.0 / (SV * SW)
                # DVE evac: ACT is the bottleneck during the attention phase
                nc.vector.tensor_scalar_mul(osb[:], ot, oscale)
                dma_eng = nc.sync if (tb + oc) % 2 == 0 else nc.scalar
                dma_eng.dma_start(
                    out=t["out"].ap()[tb * KW:(tb + 1) * KW,
                                      oc * QW:(oc + 1) * QW],
                    in_=osb[:])

        NOC = DM // QW

        def oust(j):
            return [(tb, oc) for tb in range(j * 4, (j + 1) * 4)
                    for oc in range(NOC)]

        # ---- diagonal pipeline: chunk-c projections interleave with
        #      attention for chunk c-1 (uses wave-A ACT slack for exp) ----
        for c in range(NTC):
            xtc = xt0 if c == 0 else load_chunk(c)
            units = chunk_units(c, xtc)
            if c == 0:
                for u in units:
                    u()
            else:
                n = len(units)
                for pr in range(NKV):
                    lo = pr * n // NKV
                    hi = (pr + 1) * n // NKV
                    attn_pair(pr, c - 1, fillers=units[lo:hi])
        # weights done; free their SBUF, then load Wo (fp8 hi/lo pair)
        wts.close()
        late = es.enter_context(tc.tile_pool(name="late", bufs=1))
        wo8h = late.tile([128, NQ, DM], FP8, tag="wo8h", name="wo8h")
        wo8l = late.tile([128, NQ, DM], FP8, tag="wo8l", name="wo8l")
        for h in range(NQ):
            eng = nc.sync if h % 2 == 0 else nc.scalar
            eng.dma_start(out=wo8h[:, h, :],
                          in_=t["wo8h"].ap()[h * HD:(h + 1) * HD, :])
            eng2 = nc.scalar if h % 2 == 0 else nc.sync
            eng2.dma_start(out=wo8l[:, h, :],
                           in_=t["wo8l"].ap()[h * HD:(h + 1) * HD, :])
        # j3 attention with outproj of chunks 0-2 as PE filler
        fill = oust(0) + oust(1) + oust(2)
        for pr in range(NKV):
            mine = fill[pr * 12:(pr + 1) * 12]
            attn_pair(pr, 3, fillers=[
                (lambda u=u: outproj_units([u])) for u in mine])
        flush_pend(depth=0)
        outproj_units(oust(3))


def build(TOK=S, DM=D):
    cfg = dict(TOK=TOK, DM=DM)
    nc = bacc.Bacc("TRN2", target_bir_lowering=False, debug=False)
    t = {}
    t["x8"] = nc.dram_tensor("x8", [DM, 2 * TOK], FP8, kind="ExternalInput")
    t["wqh"] = nc.dram_tensor("wqh", [DM, NQ * HD], FP8, kind="ExternalInput")
    if not Q2B:
        t["wql"] = nc.dram_tensor("wql", [DM, NQ * HD], FP8,
                                  kind="ExternalInput")
    t["wkvh"] = nc.dram_tensor("wkvh", [DM, 2 * NKV * HD], FP8,
                               kind="ExternalInput")
    t["wkvl"] = nc.dram_tensor("wkvl", [DM, 2 * NKV * HD], FP8,
                               kind="ExternalInput")
    t["wo8h"] = nc.dram_tensor("wo8h", [NQ * HD, DM], FP8,
                               kind="ExternalInput")
    t["wo8l"] = nc.dram_tensor("wo8l", [NQ * HD, DM], FP8,
                               kind="ExternalInput")
    t["cpk"] = nc.dram_tensor("cpk", [128, TOK], FP16, kind="ExternalInput")
    t["spk"] = nc.dram_tensor("spk", [128, TOK], FP16, kind="ExternalInput")
    t["negI"] = nc.dram_tensor("negI", [64, 2 * 128], FP8,
                               kind="ExternalInput")
    t["umask"] = nc.dram_tensor("umask", [64, 4 * 2 * QW], FP8,
                                kind="ExternalInput")
    t["out"] = nc.dram_tensor("out", [TOK, DM], FP16, kind="ExternalOutput")
    with tile.TileContext(nc) as tc:
        _body(nc, tc, cfg, t)
    nc.compile()
    return nc


# ---------------- host-side sharding ----------------

def _rope_perm():
    return np.concatenate([np.arange(0, 128, 2), np.arange(1, 128, 2)])


def _res(v):
    hi = v.astype(F8NP)
    lo = (v - hi.astype(np.float32)).astype(F8NP)
    return hi, lo


def _consts():
    negI = np.zeros((64, 2, 128), np.float32)
    for sl in range(2):
        for r in range(64):
            negI[r, sl, r + 64 * sl] = -15.0
    kk = np.arange(128)[:, None]
    qq = np.arange(QW)[None, :]
    umask = np.zeros((64, 4, 2, QW), np.float32)
    for di in range(4):
        u = ((di * 128 + kk) > qq) * 15.0
        umask[:, di, 0, :] = u[0:64]
        umask[:, di, 1, :] = u[64:128]
    return (negI.reshape(64, 256).astype(F8NP),
            umask.reshape(64, 4 * 2 * QW).astype(F8NP))


def shard_inputs(x, freqs_cos, freqs_sin, Wq, Wk, Wv, Wo):
    perm = _rope_perm()
    negI, umask = _consts()
    cpk = np.concatenate([freqs_cos.T, freqs_sin.T], 0).astype(np.float16)
    spk = np.concatenate([-freqs_sin.T, freqs_cos.T], 0).astype(np.float16)

    in_maps = []
    for b in range(B):
        xt = np.ascontiguousarray(np.asarray(x)[b].T).astype(np.float32) * SX
        xhi, xlo = _res(xt)
        x8 = np.empty((D, 2 * S), F8NP)
        for c in range(S // QW):
            x8[:, c * 2 * QW:c * 2 * QW + QW] = xhi[:, c * QW:(c + 1) * QW]
            x8[:, c * 2 * QW + QW:(c + 1) * 2 * QW] = (
                xlo[:, c * QW:(c + 1) * QW])
        for g in range(NG):
            qh = slice(g * NQ * HD, (g + 1) * NQ * HD)
            kvh = slice(g * NKV * HD, (g + 1) * NKV * HD)
            wq_g = (Wq[:, qh].reshape(D, NQ, HD)[:, :, perm]
                    .reshape(D, NQ * HD).astype(np.float32) * SW)
            wk_g = (Wk[:, kvh].reshape(D, NKV, HD)[:, :, perm]
                    .reshape(D, NKV * HD).astype(np.float32) * SW)
            wkv = np.concatenate([wk_g, Wv[:, kvh].astype(np.float32) * SW],
                                 axis=1)
            wqh_, wql_ = _res(np.ascontiguousarray(wq_g))
            wkvh_, wkvl_ = _res(np.ascontiguousarray(wkv))
            wo8h_, wo8l_ = _res(
                np.ascontiguousarray(Wo[qh, :]).astype(np.float32) * SW)
            im = dict(
                x8=x8, wqh=wqh_, wkvh=wkvh_, wkvl=wkvl_,
                wo8h=wo8h_, wo8l=wo8l_,
                cpk=cpk, spk=spk, negI=negI, umask=umask,
            )
            if not Q2B:
                im["wql"] = wql_
            in_maps.append(im)
    return in_maps


_NC_CACHE = {}


def kernel(x, freqs_cos, freqs_sin, Wq, Wk, Wv, Wo):
    """Full-problem entry point: full inputs in, full [B,S,D] fp32 out."""
    if "nc" not in _NC_CACHE:
        _NC_CACHE["nc"] = build()
    nc = _NC_CACHE["nc"]
    in_maps = shard_inputs(
        np.asarray(x), np.asarray(freqs_cos), np.asarray(freqs_sin),
        np.asarray(Wq), np.asarray(Wk), np.asarray(Wv), np.asarray(Wo),
    )
    res = run_bass_kernel_spmd(nc, in_maps, core_ids=list(range(N_CORES)))
    out = np.zeros((B, S, D), np.float32)
    for b in range(B):
        out[b] = (res.results[b * NG]["out"].astype(np.float32)
                  + res.results[b * NG + 1]["out"].astype(np.float32))
    return out


# revision 47
# speedup vs baseline: 1.0024x; 1.0012x over previous
"""Trainium2 Bass kernel for nn_Attention (GQA causal attention + RoPE), v3.

Full problem: x[4,2048,2048] -> attention(16 q heads / 8 kv heads, head_dim
128, llama RoPE, causal) -> out[4,2048,2048], fp32.

Sharding: core = batch*2 + head_group (tensor-parallel over heads x
data-parallel over batch). Host sums the two head-group partials per batch
(the Wo all-reduce).

Per-core kernel, mixed precision tuned to the TRN2 cost model:
- K/V projections: 3-term fp8(e4m3) residual DoubleRow matmuls
  (x ~ xhi+xlo, W ~ whi+wlo; terms hi*hi + hi*lo + lo*hi), 0.75x the
  bf16 cost. Q projection: 2-term (w_lo dropped, Q2B). x scaled by 32,
  W by 512 on host.
- RoPE: fp16 on DVE (ACT copy-with-scale evacuates PSUM at 1/16384).
- scores: fp16 matmuls into PSUM swaths, NARROWED on the diagonal
  (block di only computes q >= di*128); causal mask ADDED in PSUM by
  tiny fp8 DoubleRow bias matmuls ((-15 I) @ (15 U) = -225); fully
  masked strips are written by a start=True mask matmul emitted FIRST
  (start=True lazily zeroes the whole 2KB bank).
- softmax: exp on ACT emits fp8(e4m3) probs directly, in a 2-k-block
  paired layout; the denominator is an all-ones [128,2,128] fp8
  DoubleRow matmul on the PE (k-sum broadcast to all partitions, no
  partition reduce); fp16 reciprocal on DVE.
- PV: fp8 DoubleRow matmuls packing TWO k-blocks (256 contraction) per
  pass; v kept as a x16-scaled fp8 hi/lo pair -> 2 passes per block
  pair = 0.5x the fp16 cost (0.25x with narrowing).
- normalize writes 16*attn (fp16) into the dead q slice of rotq; out
  projection converts it lazily per 128-token strip to an fp8 hi/lo
  pair and runs 3-term fp8 DoubleRow matmuls against fp8 hi/lo Wo
  (0.75x fp16); partial outputs written fp16, host sums in fp32.
- schedule: diagonal pipeline — chunk-c projections run as PE fillers
  inside the (ACT-bound) attention pairs of chunk c-1; j3 attention is
  filled with out-projection units of chunks 0-2.
"""

import math
from contextlib import ExitStack

import numpy as np
import ml_dtypes

import concourse.bass as bass
import concourse.bass_isa as bass_isa
import concourse.mybir as mybir
import concourse.tile as tile
from concourse import bacc
from concourse.bass_utils import run_bass_kernel_spmd

F32 = mybir.dt.float32
FP16 = mybir.dt.float16
FP8 = mybir.dt.float8e4
F8NP = ml_dtypes.float8_e4m3
DR = mybir.MatmulPerfMode.DoubleRow

B, S, D = 4, 2048, 2048
H, KVH, HD = 16, 8, 128
NG = 2
NQ = H // NG           # 8 q heads per core
NKV = KVH // NG        # 4 kv heads per core
REP = NQ // NKV
N_CORES = 8
QW = 512               # q-chunk width
KW = 128               # k-block width
SX, SW = 32.0, 512.0   # host-side fp8 scales for x and W
SV = 16.0              # v (and attn) fp8 scale
SCALE = 1.0 / math.sqrt(HD)
Q2B = True             # 2-term q projection (drop w_lo term)


def _body(nc, tc, cfg, t):
    TOK = cfg["TOK"]
    DM = cfg["DM"]
    DC = DM // 128
    NP_ = DC // 2          # dc pairs
    NTC = TOK // QW        # chunks
    KB = TOK // KW         # k blocks
    KBP = KB // 2          # k block pairs
    inv = 1.0 / (SX * SW)

    with ExitStack() as es:
        P = es.enter_context(tc.tile_pool(name="persist", bufs=1))
        cpk = P.tile([128, TOK], FP16, tag="cpk", name="cpk")
        spk = P.tile([128, TOK], FP16, tag="spk", name="spk")
        negI = P.tile([64, 2, 128], FP8, tag="negI", name="negI")
        umask = P.tile([64, 4, 2, QW], FP8, tag="umask", name="umask")
        rotq = P.tile([128, NQ, TOK], FP16, tag="rotq", name="rotq")
        rotk = P.tile([128, NKV, TOK], FP16, tag="rotk", name="rotk")
        v8h = P.tile([128, KBP, 2, NKV * HD], FP8, tag="v8h", name="v8h")
        v8l = P.tile([128, KBP, 2, NKV * HD], FP8, tag="v8l", name="v8l")

        work = es.enter_context(tc.tile_pool(name="work", bufs=1))
        pp = es.enter_context(tc.tile_pool(name="pp", bufs=1, space="PSUM"))

        # ---- constant loads (cpk first: warmup + rope) ----
        nc.sync.dma_start(out=cpk[:], in_=t["cpk"].ap()[:])
        nc.scalar.dma_start(out=spk[:], in_=t["spk"].ap()[:])
        nc.scalar.dma_start(out=negI[:], in_=t["negI"].ap()[:])
        nc.scalar.dma_start(out=umask[:], in_=t["umask"].ap()[:])

        wts = ExitStack()
        WP = wts.enter_context(tc.tile_pool(name="wts", bufs=1))
        wqh = WP.tile([128, DC, NQ * HD], FP8, tag="wqh", name="wqh")
        if not Q2B:
            wql = WP.tile([128, DC, NQ * HD], FP8, tag="wql", name="wql")
        wkvh = WP.tile([128, DC, 2 * NKV * HD], FP8, tag="wkvh", name="wkvh")
        wkvl = WP.tile([128, DC, 2 * NKV * HD], FP8, tag="wkvl", name="wkvl")

        def load_chunk(c):
            xt = WP.tile([128, DC, 2 * QW], FP8, tag="xth", bufs=2, name="xt")
            ts = c * 2 * QW
            for dc in range(DC):
                eng = (nc.sync, nc.scalar, nc.gpsimd)[dc % 3]
                eng.dma_start(
                    out=xt[:, dc, :],
                    in_=t["x8"].ap()[dc * 128:(dc + 1) * 128,
                                     ts:ts + 2 * QW])
            return xt

        # PE p-state warmup: matmuls on a memset tile while loads run
        wum = pp.tile([128, 2, QW], F32, tag="swath", bufs=2, name="wum")
        wsrc = work.tile([128, 256], FP16, tag="wsrc", bufs=1, name="wsrc")
        nc.vector.memset(wsrc[:], 0.5)
        for i in range(40):
            nc.tensor.matmul(wum[:, 0, 0:128], wsrc[:, 0:128],
                             wsrc[:, 128:256],
                             start=True, stop=True, skip_group_check=True)

        xt0 = WP.tile([128, DC, 2 * QW], FP8, tag="xth", bufs=2, name="xt0")
        for dc in range(DC):
            # spread chunk-0/wkv loads across both HWDGE queues + SWDGE
            e1 = (nc.sync, nc.scalar, nc.gpsimd)[dc % 3]
            e2 = (nc.scalar, nc.gpsimd, nc.sync)[dc % 3]
            e4 = (nc.gpsimd, nc.sync, nc.scalar)[dc % 3]
            e1.dma_start(
                out=xt0[:, dc, :],
                in_=t["x8"].ap()[dc * 128:(dc + 1) * 128, 0:2 * QW])
            e2.dma_start(out=wkvh[:, dc, :],
                         in_=t["wkvh"].ap()[dc * 128:(dc + 1) * 128, :])
            e4.dma_start(
                out=wkvl[:, dc, :],
                in_=t["wkvl"].ap()[dc * 128:(dc + 1) * 128, :])
        for dc in range(DC):
            nc.sync.dma_start(out=wqh[:, dc, :],
                              in_=t["wqh"].ap()[dc * 128:(dc + 1) * 128, :])
            if not Q2B:
                nc.gpsimd.dma_start(
                    out=wql[:, dc, :],
                    in_=t["wql"].ap()[dc * 128:(dc + 1) * 128, :])

        def proj3(ps, wh, wl, xt, coff, cw, xoff=None, xw=None):
            """3-term fp8 residual projection over all dc pairs.

            xt holds [hi | lo] halves packed along the free dim.
            wl=None drops the w_lo term (2-term projection).
            """
            plan = []
            for p in range(NP_):
                dcs = slice(2 * p, 2 * p + 2)
                if xoff is None:
                    plan.append((0, wh[:, dcs, coff:coff + cw],
                                 xt[:, dcs, 0:QW]))
                    if wl is not None:
                        plan.append((1, wl[:, dcs, coff:coff + cw],
                                     xt[:, dcs, 0:QW]))
                    plan.append((1, wh[:, dcs, coff:coff + cw],
                                 xt[:, dcs, QW:2 * QW]))
                else:
                    plan.append((0, xt[:, dcs, xoff:xoff + xw],
                                 wh[:, dcs, coff:coff + cw]))
                    plan.append((1, xt[:, dcs, QW + xoff:QW + xoff + xw],
                                 wh[:, dcs, coff:coff + cw]))
                    if wl is not None:
                        plan.append((1, xt[:, dcs, xoff:xoff + xw],
                                     wl[:, dcs, coff:coff + cw]))
            plan.sort(key=lambda it: it[0])  # all hi*hi terms first
            for i, (_, a, b_) in enumerate(plan):
                nc.tensor.matmul(ps, a, b_, start=(i == 0),
                                 stop=(i == len(plan) - 1), perf_mode=DR,
                                 skip_group_check=True)

        NH = NQ + NKV  # heads per chunk-batch of rope work (12)
        NHB = NH // 6  # rope dup processed in 6 head-batches of 2

        def proj_psum():
            tl = pp.tile([128, 2, QW], F32, tag="swath", bufs=2, name="ps")
            return tl[:, 0, :]

        def proj_head(wh, wl, coff, rawAll, hh, xt):
            """Project one q/k head; evacuate into rawAll[:, hh, :]."""
            ps = proj_psum()
            proj3(ps, wh, wl, xt, coff, HD)
            nc.scalar.activation(rawAll[:, hh, :], ps,
                                 mybir.ActivationFunctionType.Copy, scale=inv)

        def rope_batch(rawAll, qeAll, qoAll, dests, ts):
            """Duplicate even/odd halves for a head-batch, then rotate."""
            nc.gpsimd.dma_start(out=qeAll[0:64, :, :], in_=rawAll[0:64, :, :])
            nc.sync.dma_start(out=qeAll[64:128, :, :],
                               in_=rawAll[0:64, :, :])
            nc.gpsimd.dma_start(out=qoAll[0:64, :, :],
                                in_=rawAll[64:128, :, :])
            nc.sync.dma_start(out=qoAll[64:128, :, :],
                               in_=rawAll[64:128, :, :])
            for i, dest in enumerate(dests):
                t1 = work.tile([128, QW], FP16, tag="t1", bufs=1, name="t1")
                t2 = work.tile([128, QW], FP16, tag="t2", bufs=1, name="t2")
                nc.vector.tensor_mul(t1[:], qeAll[:, i, :], cpk[:, ts:ts + QW])
                nc.vector.tensor_mul(t2[:], qoAll[:, i, :], spk[:, ts:ts + QW])
                nc.vector.tensor_add(dest, t1[:], t2[:])

        # ---- wave A: chunks 0..NTC-2; last chunk interleaves into
        #      the ACT-bound early attention as PE filler ----
        def chunk_units(c, xt):
            """Generate per-chunk projection work as callable units."""
            ts = c * QW
            vunits = []
            for tb in range(QW // KW):
                def vproj(tb=tb):
                    psv = proj_psum()
                    proj3(psv, wkvh, wkvl, xt, NKV * HD, NKV * HD,
                          xoff=tb * KW, xw=KW)
                    blk = c * (QW // KW) + tb
                    v16 = work.tile([128, NKV * HD], FP16, tag="v16",
                                    bufs=1, name="v16")
                    nc.vector.tensor_scalar_mul(v16[:], psv, inv * SV)
                    nc.vector.tensor_copy(v8h[:, blk // 2, blk % 2, :],
                                          v16[:])
                    nc.vector.tensor_sub(v8l[:, blk // 2, blk % 2, :],
                                         v16[:], v8h[:, blk // 2, blk % 2, :])
                vunits.append(vproj)
            units = []
            heads = ([("kv", kv) for kv in range(NKV)]
                     + [("q", h) for h in range(NQ)])
            for hb in range(6):
                def ropeu(hb=hb):
                    batch = heads[hb * NHB:(hb + 1) * NHB]
                    rawAll = work.tile([128, NHB, QW], FP16, tag="rawAll",
                                       bufs=2, name="rawAll")
                    qeAll = work.tile([128, NHB, QW], FP16, tag="qeAll",
                                      bufs=2, name="qeAll")
                    qoAll = work.tile([128, NHB, QW], FP16, tag="qoAll",
                                      bufs=2, name="qoAll")
                    dests = []
                    for i, (kind, idx) in enumerate(batch):
                        if kind == "kv":
                            proj_head(wkvh, wkvl, idx * HD, rawAll, i, xt)
                            dests.append(rotk[:, idx, ts:ts + QW])
                        else:
                            proj_head(wqh, None if Q2B else wql, idx * HD,
                                      rawAll, i, xt)
                            dests.append(rotq[:, idx, ts:ts + QW])
                    rope_batch(rawAll, qeAll, qoAll, dests, ts)
                units.append(ropeu)
            # k/q head units first (attention-critical), v projections last
            return units + vunits




        # -------- attention by head-pair, j outer; outproj interleaved ----
        # k blocks are processed in PAIRS (DoubleRow 256-deep PV contraction)
        # all-ones stationary: the denominator matmul broadcasts the k-sum
        # of e to every output partition (no partition reduce needed)
        ones8 = P.tile([128, 2, 128], FP8, tag="ones8", name="ones8")
        nc.vector.memset(ones8[:], 1.0)
        pend = []

        def _finish(pr, j, poP, denP):
            qs = j * QW
            bc = work.tile([128, 2, QW], FP16, tag="bc", bufs=1, name="bc")
            with nc.allow_low_precision(reason="softmax rec fp16"):
                nc.vector.reciprocal(bc[:], denP[:])
            # writes 16*attn into the dead q slice of rotq (v was x16)
            nc.vector.tensor_mul(rotq[:, 2 * pr:2 * pr + 2, qs:qs + QW],
                                 poP[:], bc[:])

        def flush_pend(depth=1):
            while len(pend) > depth:
                _flush_one()

        def _flush_one():
            e, pr, j, p, po_off, poP, denP = pend.pop(0)
            kh = pr
            npair = 2 * j + 2
            st, sp = (p == 0), (p == npair - 1)
            for i in range(2):
                for ti, v8 in ((0, v8h), (1, v8l)):
                    nc.tensor.matmul(
                        poP[:, i, po_off:],
                        v8[:, p, :, kh * HD:(kh + 1) * HD],
                        e[:, :, i, po_off:],
                        start=(st and ti == 0), stop=(sp and ti == 1),
                        perf_mode=DR, skip_group_check=True)
                # softmax denominator: ones.T @ e on the PE (fp8 DoubleRow)
                nc.tensor.matmul(
                    denP[:, i, po_off:], ones8[:], e[:, :, i, po_off:],
                    start=st, stop=sp, perf_mode=DR, skip_group_check=True)
            if sp:
                _finish(pr, j, poP, denP)

        def attn_pair(pr, j, fillers=(), foff=0):
            kh = pr
            qs = j * QW
            fillers = list(fillers)
            npair = 2 * j + 2
            nfp = npair - foff
            poP = eaP = None
            for p in range(2 * j + 2):
                # PE filler while ACT chews on exp: spread over pairs >= foff
                if p >= foff:
                    lo = (p - foff) * len(fillers) // nfp
                    hi = (p - foff + 1) * len(fillers) // nfp
                    for u in fillers[lo:hi]:
                        u()
                b0 = 2 * p
                # pair q-range starts at the even block's diag offset
                po_off = max(0, (b0 - 4 * j)) * KW
                swts = []
                for s in range(2):
                    b = b0 + s
                    diag = b >= 4 * j
                    ob = max(0, (b - 4 * j)) * KW
                    swt = pp.tile([128, 2, QW], F32, tag="swath", bufs=2,
                                  name="swt")
                    swts.append(swt)
                    strip = diag and ob > po_off
                    for i in range(2):
                        if strip:
                            # fully-masked strip left of this block's own
                            # range: constant -225. Must be emitted FIRST:
                            # start=True lazily zeroes the whole 2KB bank.
                            nc.tensor.matmul(
                                swt[:, i, po_off:ob], negI[:],
                                umask[:, b - 4 * j, :, po_off:ob],
                                start=True, stop=False,
                                perf_mode=DR, skip_group_check=True)
                        nc.tensor.matmul(
                            swt[:, i, ob:],
                            rotk[:, kh, b * 128:(b + 1) * 128],
                            rotq[:, 2 * pr + i, qs + ob:qs + QW],
                            start=not strip, stop=not diag,
                            skip_group_check=True)
                    if diag:
                        di = b - 4 * j
                        for i in range(2):
                            nc.tensor.matmul(
                                swt[:, i, ob:ob + KW], negI[:],
                                umask[:, di, :, ob:ob + KW],
                                start=False, stop=True,
                                perf_mode=DR, skip_group_check=True)
                e = work.tile([128, 2, 2, QW], FP8, tag="e", bufs=4,
                              name="e")
                for s in range(2):
                    nc.scalar.activation(e[:, s, :, po_off:],
                                         swts[s][:, :, po_off:],
                                         mybir.ActivationFunctionType.Exp,
                                         scale=SCALE)
                flush_pend(depth=2)
                if p == 0:
                    poP = pp.tile([128, 2, QW], F32, tag="popair", bufs=1,
                                  name="poP")
                    denP = pp.tile([128, 2, QW], F32, tag="den", bufs=1,
                                   name="denP")
                pend.append((e, pr, j, p, po_off, poP, denP))

        _last_tb = [-1, None]

        def outproj_units(units):
            for tb, oc in units:
                tsl = slice(tb * KW, (tb + 1) * KW)
                if tb != _last_tb[0]:
                    # convert this token strip of 16*attn to an fp8 pair
                    a8h = work.tile([128, NQ, KW], FP8, tag="a8h", bufs=2,
                                    name="a8h")
                    a8l = work.tile([128, NQ, KW], FP8, tag="a8l", bufs=2,
                                    name="a8l")
                    nc.vector.tensor_copy(a8h[:], rotq[:, :, tsl])
                    nc.vector.tensor_sub(a8l[:], rotq[:, :, tsl], a8h[:])
                    _last_tb[0] = tb
                    _last_tb[1] = (a8h, a8l)
                a8h, a8l = _last_tb[1]
                ots = pp.tile([128, 2, QW], F32, tag="swath", bufs=2,
                              name="ots")
                ot = ots[:, 0, :]
                osl = slice(oc * QW, (oc + 1) * QW)
                nmm = 3 * (NQ // 2)
                mi = 0
                for hp in range(NQ // 2):
                    hs = slice(2 * hp, 2 * hp + 2)
                    for a8, wo8 in ((a8h, wo8h), (a8h, wo8l), (a8l, wo8h)):
                        nc.tensor.matmul(ot, a8[:, hs, :], wo8[:, hs, osl],
                                         start=(mi == 0),
                                         stop=(mi == nmm - 1),
                                         perf_mode=DR,
                                         skip_group_check=True)
                        mi += 1
                osb = work.tile([128, QW], FP16, tag="osb", bufs=2,
                                name="osb")
                oscale = 1.0 / (SV * SW)
                # DVE evac: ACT is the bottleneck during the attention phase
                nc.vector.tensor_scalar_mul(osb[:], ot, oscale)
                dma_eng = nc.sync if (tb + oc) % 2 == 0 else nc.scalar
                dma_eng.dma_start(
                    out=t["out"].ap()[tb * KW:(tb + 1) * KW,
                                      oc * QW:(oc + 1) * QW],
                    in_=osb[:])

        NOC = DM // QW

        def oust(j):
            return [(tb, oc) for tb in range(j * 4, (j + 1) * 4)
                    for oc in range(NOC)]

        # ---- diagonal pipeline: chunk-c projections interleave with
        #      attention for chunk c-1 (uses wave-A ACT slack for exp) ----
        for c in range(NTC):
            xtc = xt0 if c == 0 else load_chunk(c)
            if c == NTC - 1:
                # prefetch the hi half of Wo during the last chunk phase
                # (fits beside the projection weights once wql is gone)
                wo8h = P.tile([128, NQ, DM], FP8, tag="wo8h",
                              name="wo8h")
                for h in range(NQ):
                    eng = nc.sync if h % 2 == 0 else nc.scalar
                    eng.dma_start(
                        out=wo8h[:, h, :],
                        in_=t["wo8h"].ap()[h * HD:(h + 1) * HD, :])
            units = chunk_units(c, xtc)
            if c == 0:
                for u in units:
                    u()
            else:
                n = len(units)
                for pr in range(NKV):
                    lo = pr * n // NKV
                    hi = (pr + 1) * n // NKV
                    attn_pair(pr, c - 1, fillers=units[lo:hi])
        # weights done; free their SBUF, then load the lo half of Wo
        wts.close()
        latel = es.enter_context(tc.tile_pool(name="latel", bufs=1))
        wo8l = latel.tile([128, NQ, DM], FP8, tag="wo8l", name="wo8l")
        for h in range(NQ):
            eng2 = nc.scalar if h % 2 == 0 else nc.sync
            eng2.dma_start(out=wo8l[:, h, :],
                           in_=t["wo8l"].ap()[h * HD:(h + 1) * HD, :])
        # j3 attention with outproj of chunks 0-2 as PE filler
        fill = oust(0) + oust(1) + oust(2)
        for pr in range(NKV):
            mine = fill[pr * 12:(pr + 1) * 12]
            attn_pair(pr, 3, fillers=[
                (lambda u=u: outproj_units([u])) for u in mine])
        flush_pend(depth=0)
        outproj_units(oust(3))


def build(TOK=S, DM=D):
    cfg = dict(TOK=TOK, DM=DM)
    nc = bacc.Bacc("TRN2", target_bir_lowering=False, debug=False)
    t = {}
    t["x8"] = nc.dram_tensor("x8", [DM, 2 * TOK], FP8, kind="ExternalInput")
    t["wqh"] = nc.dram_tensor("wqh", [DM, NQ * HD], FP8, kind="ExternalInput")
    if not Q2B:
        t["wql"] = nc.dram_tensor("wql", [DM, NQ * HD], FP8,
                                  kind="ExternalInput")
    t["wkvh"] = nc.dram_tensor("wkvh", [DM, 2 * NKV * HD], FP8,
                               kind="ExternalInput")
    t["wkvl"] = nc.dram_tensor("wkvl", [DM, 2 * NKV * HD], FP8,
                               kind="ExternalInput")
    t["wo8h"] = nc.dram_tensor("wo8h", [NQ * HD, DM], FP8,
                               kind="ExternalInput")
    t["wo8l"] = nc.dram_tensor("wo8l", [NQ * HD, DM], FP8,
                               kind="ExternalInput")
    t["cpk"] = nc.dram_tensor("cpk", [128, TOK], FP16, kind="ExternalInput")
    t["spk"] = nc.dram_tensor("spk", [128, TOK], FP16, kind="ExternalInput")
    t["negI"] = nc.dram_tensor("negI", [64, 2 * 128], FP8,
                               kind="ExternalInput")
    t["umask"] = nc.dram_tensor("umask", [64, 4 * 2 * QW], FP8,
                                kind="ExternalInput")
    t["out"] = nc.dram_tensor("out", [TOK, DM], FP16, kind="ExternalOutput")
    with tile.TileContext(nc) as tc:
        _body(nc, tc, cfg, t)
    nc.compile()
    return nc


# ---------------- host-side sharding ----------------

def _rope_perm():
    return np.concatenate([np.arange(0, 128, 2), np.arange(1, 128, 2)])


def _res(v):
    hi = v.astype(F8NP)
    lo = (v - hi.astype(np.float32)).astype(F8NP)
    return hi, lo


def _consts():
    negI = np.zeros((64, 2, 128), np.float32)
    for sl in range(2):
        for r in range(64):
            negI[r, sl, r + 64 * sl] = -15.0
    kk = np.arange(128)[:, None]
    qq = np.arange(QW)[None, :]
    umask = np.zeros((64, 4, 2, QW), np.float32)
    for di in range(4):
        u = ((di * 128 + kk) > qq) * 15.0
        umask[:, di, 0, :] = u[0:64]
        umask[:, di, 1, :] = u[64:128]
    return (negI.reshape(64, 256).astype(F8NP),
            umask.reshape(64, 4 * 2 * QW).astype(F8NP))


def shard_inputs(x, freqs_cos, freqs_sin, Wq, Wk, Wv, Wo):
    perm = _rope_perm()
    negI, umask = _consts()
    cpk = np.concatenate([freqs_cos.T, freqs_sin.T], 0).astype(np.float16)
    spk = np.concatenate([-freqs_sin.T, freqs_cos.T], 0).astype(np.float16)

    in_maps = []
    for b in range(B):
        xt = np.ascontiguousarray(np.asarray(x)[b].T).astype(np.float32) * SX
        xhi, xlo = _res(xt)
        x8 = np.empty((D, 2 * S), F8NP)
        for c in range(S // QW):
            x8[:, c * 2 * QW:c * 2 * QW + QW] = xhi[:, c * QW:(c + 1) * QW]
            x8[:, c * 2 * QW + QW:(c + 1) * 2 * QW] = (
                xlo[:, c * QW:(c + 1) * QW])
        for g in range(NG):
            qh = slice(g * NQ * HD, (g + 1) * NQ * HD)
            kvh = slice(g * NKV * HD, (g + 1) * NKV * HD)
            wq_g = (Wq[:, qh].reshape(D, NQ, HD)[:, :, perm]
                    .reshape(D, NQ * HD).astype(np.float32) * SW)
            wk_g = (Wk[:, kvh].reshape(D, NKV, HD)[:, :, perm]
                    .reshape(D, NKV * HD).astype(np.float32) * SW)
            wkv = np.concatenate([wk_g, Wv[:, kvh].astype(np.float32) * SW],
                                 axis=1)
            wqh_, wql_ = _res(np.ascontiguousarray(wq_g))
            wkvh_, wkvl_ = _res(np.ascontiguousarray(wkv))
            wo8h_, wo8l_ = _res(
                np.ascontiguousarray(Wo[qh, :]).astype(np.float32) * SW)
            im = dict(
                x8=x8, wqh=wqh_, wkvh=wkvh_, wkvl=wkvl_,
                wo8h=wo8h_, wo8l=wo8l_,
                cpk=cpk, spk=spk, negI=negI, umask=umask,
            )
            if not Q2B:
                im["wql"] = wql_
            in_maps.append(im)
    return in_maps


_NC_CACHE = {}


def kernel(x, freqs_cos, freqs_sin, Wq, Wk, Wv, Wo):
    """Full-problem entry point: full inputs in, full [B,S,D] fp32 out."""
    if "nc" not in _NC_CACHE:
        _NC_CACHE["nc"] = build()
    nc = _NC_CACHE["nc"]
    in_maps = shard_inputs(
        np.asarray(x), np.asarray(freqs_cos), np.asarray(freqs_sin),
        np.asarray(Wq), np.asarray(Wk), np.asarray(Wv), np.asarray(Wo),
    )
    res = run_bass_kernel_spmd(nc, in_maps, core_ids=list(range(N_CORES)))
    out = np.zeros((B, S, D), np.float32)
    for b in range(B):
        out[b] = (res.results[b * NG]["out"].astype(np.float32)
                  + res.results[b * NG + 1]["out"].astype(np.float32))
    return out
